# revision 1
# baseline (speedup 1.0000x reference)
"""KANLinear (RBF-KAN) Trainium2 kernel.

Math (matches the reference):
  x_flat [B=8192, IN=1024]
  base   = silu(x) @ (base_w.T) + base_b
  basis[b,i,g] = exp(-(d*(x[b,i]-grid[g]))**2),  grid = linspace(-2,2,8), d = 1/(delta+1e-6)
  spline = einsum('big,oig->bo', basis, spline_w)
  out    = base + spline        [B, OUT=1024]

Implementation:
  - Data parallel over tokens: 8 cores x 1024 tokens each; weights replicated.
  - The spline contraction is a [tok, IN*G=8192] @ [8192, OUT] matmul. Per core we
    hold spline_w (transposed to [G*IN, OUT], bf16, 16MB) resident in SBUF and run
    bf16 matmuls with K accumulated in PSUM (fp32).
  - Basis tiles are produced on the fly:
      v = (x - 2g)*x          (one VectorE scalar_tensor_tensor, fp32)
      basis = Exp(-d^2*v - d^2*g^2)   (one ScalarE activation, bf16 out)
    which equals exp(-d^2 (x-g)^2) exactly.
  - silu(x) is computed as x*(1+tanh(x/2)) (tanh lives in the same ACT table set
    as exp, avoiding table switches); the 0.5 factor is folded into base_w host-side.
  - base_b is added via a K=1 rank-1 matmul (ones row x bias row).
  - Layout: out[tokens(part), out(free)] so the result DMAs out contiguously.
"""

import os
import sys

os.environ.setdefault("MYCRO_LOCAL_CACHE", "1")
for _p in ("/opt/trn_rl_repo", "/root/.axon_site/_ro/trn_rl_repo"):
    if os.path.isdir(_p) and _p not in sys.path:
        sys.path.insert(0, _p)

import numpy as np
import ml_dtypes

IN_F = 1024
OUT_F = 1024
G = 8
GRID_LO, GRID_HI = -2.0, 2.0
NCORES = 8
TOK = 8192
TCORE = TOK // NCORES   # 1024 tokens per core
NG = 2                  # token groups per core
GTOK = TCORE // NG      # 512 tokens per group
MT = GTOK // 128        # 4 psum m-tiles (128 tokens) per group
KS = G * (IN_F // 128)  # 64 spline k-tiles
KB = IN_F // 128        # 8 base k-tiles

_DELTA = float((GRID_HI - GRID_LO) / (G - 1))
_D = 1.0 / (_DELTA + 1e-6)
# match jax's f32 linspace values
_GRID = np.linspace(GRID_LO, GRID_HI, G, dtype=np.float32).astype(np.float64)

TRACE = False
LAST_RESULT = None
_NC_CACHE = None


def build_nc(reps=1):
    from concourse import bacc
    import concourse.mybir as mybir
    import concourse.tile as tile

    F32 = mybir.dt.float32
    BF16 = mybir.dt.bfloat16
    Alu = mybir.AluOpType
    Act = mybir.ActivationFunctionType

    nc = bacc.Bacc("TRN2", target_bir_lowering=False)
    xg_d = nc.dram_tensor("xg", [NG, 128, KB, GTOK], F32, kind="ExternalInput")
    spl_d = nc.dram_tensor("spline", [KS * 128, OUT_F], BF16, kind="ExternalInput")
    bw_d = nc.dram_tensor("basew", [IN_F, OUT_F], BF16, kind="ExternalInput")
    bb_d = nc.dram_tensor("brow", [1, OUT_F], BF16, kind="ExternalInput")
    out_d = nc.dram_tensor("out", [TCORE, OUT_F], F32, kind="ExternalOutput")

    d2 = _D * _D

    # Register const APs for the per-grid Exp biases (activation() requires a
    # pre-registered [128,1] const tensor for non-trivial float biases).
    def register_const_ap(value):
        t = nc.alloc_sbuf_tensor(f"const-bias-{value}", [128, 1], F32)
        nc.gpsimd.memset(t.ap(), value)
        nc.const_aps.aps[(F32, value)] = t.ap()

    def exp_bias(g):
        gval = float(_GRID[g])
        return float(-d2 * gval * gval)

    for value in sorted({exp_bias(g) for g in range(G)}):
        register_const_ap(value)
    nc.all_engine_barrier()

    with tile.TileContext(nc) as tc:
        with (
            tc.tile_pool(name="const", bufs=1) as cpool,
            tc.tile_pool(name="xg", bufs=2) as xpool,
            tc.tile_pool(name="silu", bufs=1) as spool,
            tc.tile_pool(name="tanh", bufs=1) as tpool,
            tc.tile_pool(name="v", bufs=2) as vpool,
            tc.tile_pool(name="basis", bufs=3) as bpool,
            tc.tile_pool(name="osb", bufs=3) as opool,
            tc.tile_pool(name="psum", bufs=4, space="PSUM") as ppool,
        ):
            spl_sb = cpool.tile([128, KS, OUT_F], BF16)
            bw_sb = cpool.tile([128, KB, OUT_F], BF16)
            ones_sb = cpool.tile([1, 128], BF16)
            brow_sb = cpool.tile([1, OUT_F], BF16)
            spl_view = spl_d[:].rearrange("(k p) n -> p k n", p=128)
            bw_view = bw_d[:].rearrange("(k p) n -> p k n", p=128)

            if reps == 0:
                # minimal program used as a dispatch-overhead baseline in bench.py
                z = cpool.tile([128, OUT_F], F32, name="zrow")
                nc.vector.memset(z[:], 0.0)
                nc.sync.dma_start(out_d[0:128, :], z[:])

            pending = []  # psum tiles of the previous group awaiting eviction

            def emit_evictions():
                # split each copy across DVE+ACT so the psum slot frees fast
                for ps_t, mg in pending:
                    o = opool.tile([128, OUT_F], F32, tag="osb", name=f"o_{mg}")
                    nc.vector.tensor_copy(o[:, 0:512], ps_t[:, 0:512])
                    nc.scalar.copy(o[:, 512:1024], ps_t[:, 512:1024])
                    nc.sync.dma_start(out_d[mg * 128:(mg + 1) * 128, :], o[:])
                pending.clear()

            for rep in range(reps):
              for grp in range(NG):
                xg = xpool.tile([128, KB, GTOK], F32, tag="xg", name=f"xg_r{rep}g{grp}")
                ps = [
                    ppool.tile([128, OUT_F], F32, tag="ps", name=f"ps_g{grp}m{m}")
                    for m in range(MT)
                ]
                if grp == 0 and rep == 0:
                    # HAM warmup: keep the PE busy during the initial DMA wait
                    # so the first real matmuls run at 2.4GHz. Writes are
                    # discarded by the start=True of the first real matmul.
                    nc.vector.memset(ones_sb[:], 1.0)
                    for w in range(48):
                        nc.tensor.matmul(
                            ps[w % MT][:, 0:128], ones_sb[0:1, :], ones_sb[0:1, :],
                            start=True, stop=True,
                        )
                if grp == 0:
                    # interleave the x block and the first spline k-tiles so
                    # the PE can start within a few us; then the bulk loads
                    nc.sync.dma_start(xg[:, 0:2, :], xg_d[grp, :, 0:2, :])
                    nc.sync.dma_start(spl_sb[:, 0:1, :], spl_view[:, 0:1, :])
                    nc.sync.dma_start(xg[:, 2:4, :], xg_d[grp, :, 2:4, :])
                    nc.sync.dma_start(spl_sb[:, 1:4, :], spl_view[:, 1:4, :])
                    nc.sync.dma_start(xg[:, 4:8, :], xg_d[grp, :, 4:8, :])
                    nc.sync.dma_start(spl_sb[:, 4:8, :], spl_view[:, 4:8, :])
                    for c in range(1, 8):
                        nc.sync.dma_start(
                            spl_sb[:, c * 8:(c + 1) * 8, :],
                            spl_view[:, c * 8:(c + 1) * 8, :],
                        )
                    nc.sync.dma_start(bw_sb[:], bw_view[:])
                    nc.vector.memset(ones_sb[:], 1.0)
                    nc.sync.dma_start(brow_sb[:], bb_d[:])
                else:
                    nc.sync.dma_start(xg[:], xg_d[grp, :, :, :])
                silu = spool.tile([128, KB, GTOK], BF16)

                for k in range(KS):
                    g, i = divmod(k, KB)
                    gval = float(_GRID[g])
                    v = vpool.tile([128, GTOK], F32)
                    nc.vector.scalar_tensor_tensor(
                        v[:], xg[:, i, :], -2.0 * gval, xg[:, i, :],
                        op0=Alu.add, op1=Alu.mult,
                    )
                    basis = bpool.tile([128, GTOK], BF16)
                    nc.scalar.activation(
                        basis[:], v[:], Act.Exp,
                        bias=exp_bias(k // KB), scale=float(-d2),
                    )
                    for m in range(MT):
                        lhsT = basis[:, m * 128:(m + 1) * 128]
                        for n in range(2):
                            nc.tensor.matmul(
                                ps[m][:, n * 512:(n + 1) * 512],
                                lhsT,
                                spl_sb[:, k, n * 512:(n + 1) * 512],
                                start=(k == 0), stop=False,
                            )
                    if k == 4 and pending:
                        emit_evictions()
                    if 10 <= k <= 52 and k % 6 == 4:
                        # silu2 = x*(1+tanh(x/2)) = 2*silu(x); 0.5 folded into
                        # basew. One tile every 6th k so ACT keeps pace with
                        # the exp stream.
                        i2 = (k - 10) // 6
                        t = tpool.tile([128, GTOK], F32)
                        nc.scalar.activation(t[:], xg[:, i2, :], Act.Tanh, scale=0.5)
                        nc.vector.scalar_tensor_tensor(
                            silu[:, i2, :], t[:], 1.0, xg[:, i2, :],
                            op0=Alu.add, op1=Alu.mult,
                        )

                last = grp == NG - 1
                if not last:
                    # base phase, m-interleaved; bias via rank-1 ones x brow
                    for kb in range(KB):
                        for m in range(MT):
                            lhsT = silu[:, kb, m * 128:(m + 1) * 128]
                            for n in range(2):
                                nc.tensor.matmul(
                                    ps[m][:, n * 512:(n + 1) * 512],
                                    lhsT,
                                    bw_sb[:, kb, n * 512:(n + 1) * 512],
                                    start=False, stop=False,
                                )
                    for m in range(MT):
                        for n in range(2):
                            nc.tensor.matmul(
                                ps[m][:, n * 512:(n + 1) * 512],
                                ones_sb[0:1, :],
                                brow_sb[0:1, n * 512:(n + 1) * 512],
                                start=False, stop=True,
                            )
                        pending.append((ps[m], grp * MT + m))
                else:
                    # last group: finish one m-tile at a time so evictions
                    # overlap the remaining base matmuls instead of the tail
                    for m in range(MT):
                        for kb in range(KB):
                            lhsT = silu[:, kb, m * 128:(m + 1) * 128]
                            for n in range(2):
                                nc.tensor.matmul(
                                    ps[m][:, n * 512:(n + 1) * 512],
                                    lhsT,
                                    bw_sb[:, kb, n * 512:(n + 1) * 512],
                                    start=False, stop=False,
                                )
                        for n in range(2):
                            nc.tensor.matmul(
                                ps[m][:, n * 512:(n + 1) * 512],
                                ones_sb[0:1, :],
                                brow_sb[0:1, n * 512:(n + 1) * 512],
                                start=False, stop=True,
                            )
                        pending.append((ps[m], grp * MT + m))
                        emit_evictions()
            emit_evictions()

    nc.compile()
    return nc


def _host_prep(x, base_w, base_b, spline_w):
    x = np.asarray(x, dtype=np.float32)
    base_w = np.asarray(base_w, dtype=np.float32)
    base_b = np.asarray(base_b, dtype=np.float32)
    spline_w = np.asarray(spline_w, dtype=np.float32)

    x_flat = np.ascontiguousarray(x.reshape(TOK, IN_F))
    # [OUT, IN, G] -> [G, IN, OUT] -> [G*IN, OUT]; row r = g*IN + i
    spl = np.ascontiguousarray(spline_w.transpose(2, 1, 0).reshape(G * IN_F, OUT_F))
    spl = spl.astype(ml_dtypes.bfloat16)
    bw = np.ascontiguousarray(0.5 * base_w.T).astype(ml_dtypes.bfloat16)
    brow = np.ascontiguousarray(base_b.reshape(1, OUT_F)).astype(ml_dtypes.bfloat16)

    in_maps = []
    for c in range(NCORES):
        shard = x_flat[c * TCORE:(c + 1) * TCORE, :]   # [tok, in]
        xT = shard.T                                    # [in, tok]
        # [in, tok] -> [i, p, grp, t] -> [grp, p, i, t]
        xg = np.ascontiguousarray(
            xT.reshape(KB, 128, NG, GTOK).transpose(2, 1, 0, 3)
        )
        in_maps.append({"xg": xg, "spline": spl, "basew": bw, "brow": brow})
    return in_maps


def kernel(x, base_w, base_b, spline_w):
    global _NC_CACHE, LAST_RESULT
    from concourse.bass_utils import run_bass_kernel_spmd

    in_maps = _host_prep(x, base_w, base_b, spline_w)
    if _NC_CACHE is None:
        _NC_CACHE = build_nc()
    res = run_bass_kernel_spmd(
        _NC_CACHE, in_maps, core_ids=list(range(NCORES)), trace=TRACE
    )
    LAST_RESULT = res
    outs = [np.asarray(r["out"]) for r in res.results]
    full = np.concatenate(outs, axis=0)  # [8192, 1024]
    return full.reshape(4, 2048, OUT_F)



# revision 2
# speedup vs baseline: 1.1958x; 1.1958x over previous
"""KANLinear (RBF-KAN) Trainium2 kernel.

Math (matches the reference):
  x_flat [B=8192, IN=1024]
  base   = silu(x) @ (base_w.T) + base_b
  basis[b,i,g] = exp(-(d*(x[b,i]-grid[g]))**2),  grid = linspace(-2,2,8), d = 1/(delta+1e-6)
  spline = einsum('big,oig->bo', basis, spline_w)
  out    = base + spline        [B, OUT=1024]

Implementation:
  - Data parallel over tokens: 8 cores x 1024 tokens each; weights replicated.
  - The spline contraction is a [tok, IN*G=8192] @ [8192, OUT] matmul. Per core we
    hold spline_w (transposed to [G*IN, OUT], bf16, 16MB) resident in SBUF and run
    bf16 matmuls with K accumulated in PSUM (fp32).
  - Basis tiles are produced on the fly:
      v = (x - 2g)*x          (one VectorE scalar_tensor_tensor, fp32)
      basis = Exp(-d^2*v - d^2*g^2)   (one ScalarE activation, bf16 out)
    which equals exp(-d^2 (x-g)^2) exactly.
  - silu(x) is computed as x*(1+tanh(x/2)) (tanh lives in the same ACT table set
    as exp, avoiding table switches); the 0.5 factor is folded into base_w host-side.
  - base_b is added via a K=1 rank-1 matmul (ones row x bias row).
  - Layout: out[tokens(part), out(free)] so the result DMAs out contiguously.
  - Per m-tile epilogue in EVERY group: base matmuls, bias, then immediate
    psum->sbuf eviction (DVE low half / ACT high half) + output DMA. This keeps
    all psum banks free by the time the next group's k=0 matmuls need them.
  - The `ones` row for warmup/bias matmuls is memset in the pre-tile preamble
    (gpsimd) so the HAM warmup matmuls depend only on the PE preamble.
"""

import os
import sys

os.environ.setdefault("MYCRO_LOCAL_CACHE", "1")
for _p in ("/opt/trn_rl_repo", "/root/.axon_site/_ro/trn_rl_repo"):
    if os.path.isdir(_p) and _p not in sys.path:
        sys.path.insert(0, _p)

import numpy as np
import ml_dtypes

IN_F = 1024
OUT_F = 1024
G = 8
GRID_LO, GRID_HI = -2.0, 2.0
NCORES = 8
TOK = 8192
TCORE = TOK // NCORES   # 1024 tokens per core
NG = 2                  # token groups per core
GTOK = TCORE // NG      # 512 tokens per group
MT = GTOK // 128        # 4 psum m-tiles (128 tokens) per group
KS = G * (IN_F // 128)  # 64 spline k-tiles
KB = IN_F // 128        # 8 base k-tiles
WARMUP = 48             # HAM warmup matmuls

_DELTA = float((GRID_HI - GRID_LO) / (G - 1))
_D = 1.0 / (_DELTA + 1e-6)
# match jax's f32 linspace values
_GRID = np.linspace(GRID_LO, GRID_HI, G, dtype=np.float32).astype(np.float64)

TRACE = False
LAST_RESULT = None
_NC_CACHE = None


def build_nc(reps=1):
    from concourse import bacc
    import concourse.mybir as mybir
    import concourse.tile as tile

    F32 = mybir.dt.float32
    BF16 = mybir.dt.bfloat16
    Alu = mybir.AluOpType
    Act = mybir.ActivationFunctionType

    nc = bacc.Bacc("TRN2", target_bir_lowering=False)
    xg_d = nc.dram_tensor("xg", [NG, 128, KB, GTOK], F32, kind="ExternalInput")
    spl_d = nc.dram_tensor("spline", [KS * 128, OUT_F], BF16, kind="ExternalInput")
    bw_d = nc.dram_tensor("basew", [IN_F, OUT_F], BF16, kind="ExternalInput")
    bb_d = nc.dram_tensor("brow", [1, OUT_F], BF16, kind="ExternalInput")
    out_d = nc.dram_tensor("out", [TCORE, OUT_F], F32, kind="ExternalOutput")

    d2 = _D * _D

    # Register const APs for the per-grid Exp biases (activation() requires a
    # pre-registered [128,1] const tensor for non-trivial float biases).
    def register_const_ap(value):
        t = nc.alloc_sbuf_tensor(f"const-bias-{value}", [128, 1], F32)
        nc.gpsimd.memset(t.ap(), value)
        nc.const_aps.aps[(F32, value)] = t.ap()

    def exp_bias(g):
        gval = float(_GRID[g])
        return float(-d2 * gval * gval)

    for value in sorted({exp_bias(g) for g in range(G)}):
        register_const_ap(value)
    # ones row for HAM warmup + rank-1 bias matmuls: memset in the preamble so
    # the warmup matmuls can start as soon as the PE preamble is done.
    ones_t = nc.alloc_sbuf_tensor("ones-row", [1, 128], BF16)
    nc.gpsimd.memset(ones_t.ap(), 1.0)
    ones_ap = ones_t.ap()
    nc.all_engine_barrier()

    with tile.TileContext(nc) as tc:
        with (
            tc.tile_pool(name="const", bufs=1) as cpool,
            tc.tile_pool(name="xg", bufs=2) as xpool,
            tc.tile_pool(name="silu", bufs=1) as spool,
            tc.tile_pool(name="tanh", bufs=1) as tpool,
            tc.tile_pool(name="v", bufs=3) as vpool,
            tc.tile_pool(name="basis", bufs=5) as bpool,
            tc.tile_pool(name="osb", bufs=2) as opool,
            tc.tile_pool(name="psum", bufs=4, space="PSUM") as ppool,
        ):
            spl_sb = cpool.tile([128, KS, OUT_F], BF16)
            bw_sb = cpool.tile([128, KB, OUT_F], BF16)
            brow_sb = cpool.tile([1, OUT_F], BF16)
            spl_view = spl_d[:].rearrange("(k p) n -> p k n", p=128)
            bw_view = bw_d[:].rearrange("(k p) n -> p k n", p=128)

            if reps == 0:
                # minimal program used as a dispatch-overhead baseline in bench.py
                z = cpool.tile([128, OUT_F], F32, name="zrow")
                nc.vector.memset(z[:], 0.0)
                nc.sync.dma_start(out_d[0:128, :], z[:])

            for rep in range(reps):
              for grp in range(NG):
                xg = xpool.tile([128, KB, GTOK], F32, tag="xg", name=f"xg_r{rep}g{grp}")
                ps = [
                    ppool.tile([128, OUT_F], F32, tag="ps", name=f"ps_g{grp}m{m}")
                    for m in range(MT)
                ]
                if grp == 0 and rep == 0:
                    # HAM warmup: keep the PE busy during the initial DMA wait
                    # so the first real matmuls run at 2.4GHz. Writes are
                    # discarded by the start=True of the first real matmul.
                    for w in range(WARMUP):
                        nc.tensor.matmul(
                            ps[w % MT][:, 0:128], ones_ap, ones_ap,
                            start=True, stop=True,
                        )
                if grp == 0:
                    # interleave the x block and the first spline k-tiles so
                    # the PE can start within a few us; then the bulk loads
                    nc.sync.dma_start(xg[:, 0:2, :], xg_d[grp, :, 0:2, :])
                    nc.sync.dma_start(spl_sb[:, 0:1, :], spl_view[:, 0:1, :])
                    nc.sync.dma_start(xg[:, 2:4, :], xg_d[grp, :, 2:4, :])
                    nc.sync.dma_start(spl_sb[:, 1:4, :], spl_view[:, 1:4, :])
                    nc.sync.dma_start(xg[:, 4:8, :], xg_d[grp, :, 4:8, :])
                    nc.sync.dma_start(spl_sb[:, 4:8, :], spl_view[:, 4:8, :])
                    for c in range(1, 8):
                        nc.sync.dma_start(
                            spl_sb[:, c * 8:(c + 1) * 8, :],
                            spl_view[:, c * 8:(c + 1) * 8, :],
                        )
                    nc.sync.dma_start(bw_sb[:], bw_view[:])
                    nc.sync.dma_start(brow_sb[:], bb_d[:])
                else:
                    nc.sync.dma_start(xg[:], xg_d[grp, :, :, :])
                silu = spool.tile([128, KB, GTOK], BF16)

                for k in range(KS):
                    g, i = divmod(k, KB)
                    gval = float(_GRID[g])
                    v = vpool.tile([128, GTOK], F32)
                    nc.vector.scalar_tensor_tensor(
                        v[:], xg[:, i, :], -2.0 * gval, xg[:, i, :],
                        op0=Alu.add, op1=Alu.mult,
                    )
                    basis = bpool.tile([128, GTOK], BF16)
                    nc.scalar.activation(
                        basis[:], v[:], Act.Exp,
                        bias=exp_bias(k // KB), scale=float(-d2),
                    )
                    for m in range(MT):
                        lhsT = basis[:, m * 128:(m + 1) * 128]
                        for n in range(2):
                            nc.tensor.matmul(
                                ps[m][:, n * 512:(n + 1) * 512],
                                lhsT,
                                spl_sb[:, k, n * 512:(n + 1) * 512],
                                start=(k == 0), stop=False,
                            )
                    if 10 <= k <= 52 and k % 6 == 4:
                        # silu2 = x*(1+tanh(x/2)) = 2*silu(x); 0.5 folded into
                        # basew. One tile every 6th k so ACT keeps pace with
                        # the exp stream.
                        i2 = (k - 10) // 6
                        t = tpool.tile([128, GTOK], F32)
                        nc.scalar.activation(t[:], xg[:, i2, :], Act.Tanh, scale=0.5)
                        nc.vector.scalar_tensor_tensor(
                            silu[:, i2, :], t[:], 1.0, xg[:, i2, :],
                            op0=Alu.add, op1=Alu.mult,
                        )

                # base phase: finish one m-tile at a time (base k-loop, bias,
                # then immediate psum->sbuf eviction + output DMA) so the psum
                # banks are all free again by the next group's k=0 matmuls and
                # evictions overlap the remaining base matmuls.
                for m in range(MT):
                    for kb in range(KB):
                        lhsT = silu[:, kb, m * 128:(m + 1) * 128]
                        for n in range(2):
                            nc.tensor.matmul(
                                ps[m][:, n * 512:(n + 1) * 512],
                                lhsT,
                                bw_sb[:, kb, n * 512:(n + 1) * 512],
                                start=False, stop=False,
                            )
                    for n in range(2):
                        nc.tensor.matmul(
                            ps[m][:, n * 512:(n + 1) * 512],
                            ones_ap,
                            brow_sb[0:1, n * 512:(n + 1) * 512],
                            start=False, stop=True,
                        )
                    mg = grp * MT + m
                    o = opool.tile([128, OUT_F], F32, tag="osb", name=f"o_{mg}")
                    nc.vector.tensor_copy(o[:, 0:512], ps[m][:, 0:512])
                    nc.scalar.copy(o[:, 512:1024], ps[m][:, 512:1024])
                    nc.sync.dma_start(out_d[mg * 128:(mg + 1) * 128, :], o[:])

    nc.compile()
    return nc


def _host_prep(x, base_w, base_b, spline_w):
    x = np.asarray(x, dtype=np.float32)
    base_w = np.asarray(base_w, dtype=np.float32)
    base_b = np.asarray(base_b, dtype=np.float32)
    spline_w = np.asarray(spline_w, dtype=np.float32)

    x_flat = np.ascontiguousarray(x.reshape(TOK, IN_F))
    # [OUT, IN, G] -> [G, IN, OUT] -> [G*IN, OUT]; row r = g*IN + i
    spl = np.ascontiguousarray(spline_w.transpose(2, 1, 0).reshape(G * IN_F, OUT_F))
    spl = spl.astype(ml_dtypes.bfloat16)
    bw = np.ascontiguousarray(0.5 * base_w.T).astype(ml_dtypes.bfloat16)
    brow = np.ascontiguousarray(base_b.reshape(1, OUT_F)).astype(ml_dtypes.bfloat16)

    in_maps = []
    for c in range(NCORES):
        shard = x_flat[c * TCORE:(c + 1) * TCORE, :]   # [tok, in]
        xT = shard.T                                    # [in, tok]
        # [in, tok] -> [i, p, grp, t] -> [grp, p, i, t]
        xg = np.ascontiguousarray(
            xT.reshape(KB, 128, NG, GTOK).transpose(2, 1, 0, 3)
        )
        in_maps.append({"xg": xg, "spline": spl, "basew": bw, "brow": brow})
    return in_maps


def kernel(x, base_w, base_b, spline_w):
    global _NC_CACHE, LAST_RESULT
    from concourse.bass_utils import run_bass_kernel_spmd

    in_maps = _host_prep(x, base_w, base_b, spline_w)
    if _NC_CACHE is None:
        _NC_CACHE = build_nc()
    res = run_bass_kernel_spmd(
        _NC_CACHE, in_maps, core_ids=list(range(NCORES)), trace=TRACE
    )
    LAST_RESULT = res
    outs = [np.asarray(r["out"]) for r in res.results]
    full = np.concatenate(outs, axis=0)  # [8192, 1024]
    return full.reshape(4, 2048, OUT_F)


# revision 6
# speedup vs baseline: 1.4897x; 1.2457x over previous
"""KANLinear (RBF-KAN) Trainium2 kernel.

Math (matches the reference):
  x_flat [B=8192, IN=1024]
  base   = silu(x) @ (base_w.T) + base_b
  basis[b,i,g] = exp(-(d*(x[b,i]-grid[g]))**2),  grid = linspace(-2,2,8), d = 1/(delta+1e-6)
  spline = einsum('big,oig->bo', basis, spline_w)
  out    = base + spline        [B, OUT=1024]

Implementation:
  - Data parallel over tokens: 8 cores x 1024 tokens each; weights replicated.
  - The spline contraction is a [tok, IN*G=8192] @ [8192, OUT] matmul with K
    accumulated in PSUM (fp32). Mixed precision over the grid dimension:
      * inner grids g in {2,3,4,5} (|grid| <= 0.86, ~88% of the spline energy
        under x~N(0,1)): bf16 operands, 32 k-tiles per group.
      * outer grids g in {0,1,6,7}: fp8 e4m3 with DoubleRow perf mode (2
        k-tiles contracted per matmul), 16 pair-steps per group. Their small
        basis mass keeps the fp8 quantization error ~1.5e-2 total. Weights are
        scaled x4 host-side (out of the e4m3 denormal range); the matching
        1/4 on the basis is folded into the Exp activation bias.
  - Basis tiles are produced on the fly:
      v = (x - 2g)*x          (one scalar_tensor_tensor, fp32; VectorE, with
                               the fp8-pair second tile on GpSimd/Pool)
      basis = Exp(-d^2*v - d^2*g^2 [- ln 4])   (ScalarE, bf16/fp8 out)
    which equals exp(-d^2 (x-g)^2) [/4] exactly.
  - silu(x) is computed as x*(1+tanh(x/2)): tanh on ScalarE (same ACT table
    set as exp), the multiply-add on GpSimd/Pool; 0.5 folded into base_w.
  - base_b is added via a K=1 rank-1 matmul (ones row x bias row); the ones
    row is memset in the pre-tile preamble so HAM-warmup matmuls start as
    soon as the PE preamble finishes.
  - Per m-tile epilogue in EVERY group: base matmuls, bias, then immediate
    psum->sbuf eviction (DVE low half / ACT high half) + output DMA, keeping
    all psum banks free by the next group's first matmuls.
"""

import os
import sys

os.environ.setdefault("MYCRO_LOCAL_CACHE", "1")
for _p in ("/opt/trn_rl_repo", "/root/.axon_site/_ro/trn_rl_repo"):
    if os.path.isdir(_p) and _p not in sys.path:
        sys.path.insert(0, _p)

import numpy as np
import ml_dtypes

IN_F = 1024
OUT_F = 1024
G = 8
GRID_LO, GRID_HI = -2.0, 2.0
NCORES = 8
TOK = 8192
TCORE = TOK // NCORES   # 1024 tokens per core
NG = 2                  # token groups per core
GTOK = TCORE // NG      # 512 tokens per group
MT = GTOK // 128        # 4 psum m-tiles (128 tokens) per group
KB = IN_F // 128        # 8 k-tiles per grid / base k-tiles
WARMUP = 56             # HAM warmup matmuls

BF_G = (2, 3, 4, 5)     # bf16 grids (inner)
FP8_PAIRS = ((0, 1), (6, 7))  # fp8 DoubleRow grid pairs (outer)
K16 = len(BF_G) * KB    # 32 bf16 k-tiles
NQ8 = 2 * KB * 2        # 32 fp8 k-tiles (2 pairs x 8 i x 2 j)
FP8_SCALE = 4.0         # host: W*4; chip: basis/4 via exp bias

_DELTA = float((GRID_HI - GRID_LO) / (G - 1))
_D = 1.0 / (_DELTA + 1e-6)
# match jax's f32 linspace values
_GRID = np.linspace(GRID_LO, GRID_HI, G, dtype=np.float32).astype(np.float64)

TRACE = False
LAST_RESULT = None
_NC_CACHE = None


def build_nc(reps=1):
    from concourse import bacc
    import concourse.mybir as mybir
    import concourse.tile as tile

    F32 = mybir.dt.float32
    BF16 = mybir.dt.bfloat16
    F8 = mybir.dt.float8e4
    Alu = mybir.AluOpType
    Act = mybir.ActivationFunctionType
    DR = mybir.MatmulPerfMode.DoubleRow

    nc = bacc.Bacc("TRN2", target_bir_lowering=False)
    xg_d = nc.dram_tensor("xg", [NG, 128, KB, GTOK], F32, kind="ExternalInput")
    spl_d = nc.dram_tensor("spline", [K16 * 128, OUT_F], BF16, kind="ExternalInput")
    spl8_d = nc.dram_tensor("spline8", [NQ8 * 128, OUT_F], F8, kind="ExternalInput")
    bw_d = nc.dram_tensor("basew", [IN_F, OUT_F], BF16, kind="ExternalInput")
    bb_d = nc.dram_tensor("brow", [1, OUT_F], BF16, kind="ExternalInput")
    out_d = nc.dram_tensor("out", [TCORE, OUT_F], F32, kind="ExternalOutput")

    d2 = _D * _D

    # Register const APs for the per-grid Exp biases (activation() requires a
    # pre-registered [128,1] const tensor for non-trivial float biases).
    def register_const_ap(value):
        t = nc.alloc_sbuf_tensor(f"const-bias-{value}", [128, 1], F32)
        nc.gpsimd.memset(t.ap(), value)
        nc.const_aps.aps[(F32, value)] = t.ap()

    def exp_bias(g, fp8=False):
        gval = float(_GRID[g])
        b = -d2 * gval * gval
        if fp8:
            b -= float(np.log(FP8_SCALE))
        return float(b)

    bias_vals = {exp_bias(g) for g in BF_G}
    bias_vals |= {exp_bias(g, fp8=True) for p in FP8_PAIRS for g in p}
    for value in sorted(bias_vals):
        register_const_ap(value)
    # ones row for HAM warmup + rank-1 bias matmuls
    ones_t = nc.alloc_sbuf_tensor("ones-row", [1, 128], BF16)
    nc.gpsimd.memset(ones_t.ap(), 1.0)
    ones_ap = ones_t.ap()
    nc.all_engine_barrier()

    with tile.TileContext(nc) as tc:
        with (
            tc.tile_pool(name="const", bufs=1) as cpool,
            tc.tile_pool(name="xg", bufs=2) as xpool,
            tc.tile_pool(name="silu", bufs=1) as spool,
            tc.tile_pool(name="tanh", bufs=2) as tpool,
            tc.tile_pool(name="v", bufs=4) as vpool,
            tc.tile_pool(name="basis", bufs=5) as bpool,
            tc.tile_pool(name="b8", bufs=3) as b8pool,
            tc.tile_pool(name="osb", bufs=3) as opool,
            tc.tile_pool(name="psum", bufs=4, space="PSUM") as ppool,
        ):
            spl_sb = cpool.tile([128, K16, OUT_F], BF16)
            spl8_sb = cpool.tile([128, NQ8, OUT_F], F8)
            bw_sb = cpool.tile([128, KB, OUT_F], BF16)
            brow_sb = cpool.tile([1, OUT_F], BF16)
            spl_view = spl_d[:].rearrange("(k p) n -> p k n", p=128)
            spl8_view = spl8_d[:].rearrange("(k p) n -> p k n", p=128)
            bw_view = bw_d[:].rearrange("(k p) n -> p k n", p=128)

            if reps == 0:
                # minimal program used as a dispatch-overhead baseline
                z = cpool.tile([128, OUT_F], F32, name="zrow")
                nc.vector.memset(z[:], 0.0)
                nc.sync.dma_start(out_d[0:128, :], z[:])

            for rep in range(reps):
              for grp in range(NG):
                xg = xpool.tile([128, KB, GTOK], F32, tag="xg", name=f"xg_r{rep}g{grp}")
                ps = [
                    ppool.tile([128, OUT_F], F32, tag="ps", name=f"ps_g{grp}m{m}")
                    for m in range(MT)
                ]
                if grp == 0 and rep == 0:
                    # HAM warmup: keep the PE busy during the initial DMA wait
                    # so the first real matmuls run at 2.4GHz. Writes are
                    # discarded by the start=True of the first real matmul.
                    for w in range(WARMUP):
                        nc.tensor.matmul(
                            ps[w % MT][:, 0:128], ones_ap, ones_ap,
                            start=True, stop=True,
                        )
                if grp == 0:
                    # interleave the x block and the first spline k-tiles so
                    # the PE can start within a few us; then the bulk loads
                    nc.sync.dma_start(xg[:, 0:2, :], xg_d[grp, :, 0:2, :])
                    nc.sync.dma_start(spl_sb[:, 0:1, :], spl_view[:, 0:1, :])
                    nc.sync.dma_start(xg[:, 2:4, :], xg_d[grp, :, 2:4, :])
                    nc.sync.dma_start(spl_sb[:, 1:4, :], spl_view[:, 1:4, :])
                    nc.sync.dma_start(xg[:, 4:8, :], xg_d[grp, :, 4:8, :])
                    nc.sync.dma_start(spl_sb[:, 4:8, :], spl_view[:, 4:8, :])
                    for c in range(1, 4):
                        nc.sync.dma_start(
                            spl_sb[:, c * 8:(c + 1) * 8, :],
                            spl_view[:, c * 8:(c + 1) * 8, :],
                        )
                    for c in range(4):
                        nc.sync.dma_start(
                            spl8_sb[:, c * 8:(c + 1) * 8, :],
                            spl8_view[:, c * 8:(c + 1) * 8, :],
                        )
                    nc.sync.dma_start(bw_sb[:], bw_view[:])
                    nc.sync.dma_start(brow_sb[:], bb_d[:])
                else:
                    nc.sync.dma_start(xg[:], xg_d[grp, :, :, :])
                silu = spool.tile([128, KB, GTOK], BF16)

                # ---- bf16 phase: inner grids, 32 k-tiles ----
                for k16 in range(K16):
                    g = BF_G[k16 // KB]
                    i = k16 % KB
                    gval = float(_GRID[g])
                    v = vpool.tile([128, GTOK], F32)
                    nc.vector.scalar_tensor_tensor(
                        v[:], xg[:, i, :], -2.0 * gval, xg[:, i, :],
                        op0=Alu.add, op1=Alu.mult,
                    )
                    basis = bpool.tile([128, GTOK], BF16)
                    nc.scalar.activation(
                        basis[:], v[:], Act.Exp,
                        bias=exp_bias(g), scale=float(-d2),
                    )
                    for m in range(MT):
                        lhsT = basis[:, m * 128:(m + 1) * 128]
                        for n in range(2):
                            nc.tensor.matmul(
                                ps[m][:, n * 512:(n + 1) * 512],
                                lhsT,
                                spl_sb[:, k16, n * 512:(n + 1) * 512],
                                start=(k16 == 0), stop=False,
                            )
                    if k16 % 4 == 1:
                        # silu2 = x*(1+tanh(x/2)) = 2*silu(x); 0.5 folded into
                        # basew. tanh here; the multiply-add runs on Pool two
                        # k-tiles later so it never delays the v stream.
                        j = k16 // 4
                        t = tpool.tile([128, GTOK], F32, tag="tanh")
                        nc.scalar.activation(t[:], xg[:, j, :], Act.Tanh, scale=0.5)
                        _pending_tanh = (j, t)
                    if k16 % 4 == 3:
                        j, t = _pending_tanh
                        nc.vector.scalar_tensor_tensor(
                            silu[:, j, :], t[:], 1.0, xg[:, j, :],
                            op0=Alu.add, op1=Alu.mult,
                        )

                # ---- fp8 phase: outer grid pairs, DoubleRow ----
                for step in range(2 * KB):
                    pi, i = divmod(step, KB)
                    b8 = b8pool.tile([128, 2, GTOK], F8)
                    for j in range(2):
                        g = FP8_PAIRS[pi][j]
                        gval = float(_GRID[g])
                        v = vpool.tile([128, GTOK], F32)
                        nc.vector.scalar_tensor_tensor(
                            v[:], xg[:, i, :], -2.0 * gval, xg[:, i, :],
                            op0=Alu.add, op1=Alu.mult,
                        )
                        nc.scalar.activation(
                            b8[:, j, :], v[:], Act.Exp,
                            bias=exp_bias(g, fp8=True), scale=float(-d2),
                        )
                    q = (pi * KB + i) * 2
                    for m in range(MT):
                        lhsT = b8[:, :, m * 128:(m + 1) * 128]
                        for n in range(2):
                            nc.tensor.matmul(
                                ps[m][:, n * 512:(n + 1) * 512],
                                lhsT,
                                spl8_sb[:, q:q + 2, n * 512:(n + 1) * 512],
                                start=False, stop=False,
                                perf_mode=DR,
                            )

                # ---- base phase: per m-tile base matmuls, bias, eviction ----
                for m in range(MT):
                    for kb in range(KB):
                        lhsT = silu[:, kb, m * 128:(m + 1) * 128]
                        for n in range(2):
                            nc.tensor.matmul(
                                ps[m][:, n * 512:(n + 1) * 512],
                                lhsT,
                                bw_sb[:, kb, n * 512:(n + 1) * 512],
                                start=False, stop=False,
                            )
                    for n in range(2):
                        nc.tensor.matmul(
                            ps[m][:, n * 512:(n + 1) * 512],
                            ones_ap,
                            brow_sb[0:1, n * 512:(n + 1) * 512],
                            start=False, stop=True,
                        )
                    mg = grp * MT + m
                    o = opool.tile([128, OUT_F], F32, tag="osb", name=f"o_{mg}")
                    nc.vector.tensor_copy(o[:, 0:512], ps[m][:, 0:512])
                    nc.scalar.copy(o[:, 512:1024], ps[m][:, 512:1024])
                    if grp == NG - 1 and m == MT - 1:
                        # shorten the tail: ship each half as soon as its copy
                        # is done
                        nc.sync.dma_start(
                            out_d[mg * 128:(mg + 1) * 128, 0:512], o[:, 0:512]
                        )
                        nc.sync.dma_start(
                            out_d[mg * 128:(mg + 1) * 128, 512:1024], o[:, 512:1024]
                        )
                    else:
                        nc.sync.dma_start(out_d[mg * 128:(mg + 1) * 128, :], o[:])

    nc.compile()
    return nc


def _host_prep(x, base_w, base_b, spline_w):
    x = np.asarray(x, dtype=np.float32)
    base_w = np.asarray(base_w, dtype=np.float32)
    base_b = np.asarray(base_b, dtype=np.float32)
    spline_w = np.asarray(spline_w, dtype=np.float32)

    x_flat = np.ascontiguousarray(x.reshape(TOK, IN_F))
    # [OUT, IN, G] -> [G, IN, OUT]; row of tile k is g*IN + i
    spl_gio = spline_w.transpose(2, 1, 0)  # [G, IN, OUT]
    spl16 = np.ascontiguousarray(
        spl_gio[list(BF_G)].reshape(K16 * 128, OUT_F)
    ).astype(ml_dtypes.bfloat16)
    # fp8 part: [pair, i, j, 128, OUT] with j indexing the two grids of the
    # pair (DoubleRow contracts over the j dimension)
    blocks = []
    for (ga, gb) in FP8_PAIRS:
        a = spl_gio[ga].reshape(KB, 128, OUT_F)
        b = spl_gio[gb].reshape(KB, 128, OUT_F)
        blocks.append(np.stack([a, b], axis=1))  # [KB, 2, 128, OUT]
    spl8 = np.concatenate(blocks, axis=0).reshape(NQ8 * 128, OUT_F)
    spl8 = np.ascontiguousarray(spl8 * FP8_SCALE).astype(ml_dtypes.float8_e4m3)
    bw = np.ascontiguousarray(0.5 * base_w.T).astype(ml_dtypes.bfloat16)
    brow = np.ascontiguousarray(base_b.reshape(1, OUT_F)).astype(ml_dtypes.bfloat16)

    in_maps = []
    for c in range(NCORES):
        shard = x_flat[c * TCORE:(c + 1) * TCORE, :]   # [tok, in]
        xT = shard.T                                    # [in, tok]
        # [in, tok] -> [i, p, grp, t] -> [grp, p, i, t]
        xg = np.ascontiguousarray(
            xT.reshape(KB, 128, NG, GTOK).transpose(2, 1, 0, 3)
        )
        in_maps.append({
            "xg": xg, "spline": spl16, "spline8": spl8,
            "basew": bw, "brow": brow,
        })
    return in_maps


def kernel(x, base_w, base_b, spline_w):
    global _NC_CACHE, LAST_RESULT
    from concourse.bass_utils import run_bass_kernel_spmd

    in_maps = _host_prep(x, base_w, base_b, spline_w)
    if _NC_CACHE is None:
        _NC_CACHE = build_nc()
    res = run_bass_kernel_spmd(
        _NC_CACHE, in_maps, core_ids=list(range(NCORES)), trace=TRACE
    )
    LAST_RESULT = res
    outs = [np.asarray(r["out"]) for r in res.results]
    full = np.concatenate(outs, axis=0)  # [8192, 1024]
    return full.reshape(4, 2048, OUT_F)


# revision 11
# speedup vs baseline: 1.4928x; 1.0021x over previous
"""KANLinear (RBF-KAN) Trainium2 kernel.

Math (matches the reference):
  x_flat [B=8192, IN=1024]
  base   = silu(x) @ (base_w.T) + base_b
  basis[b,i,g] = exp(-(d*(x[b,i]-grid[g]))**2),  grid = linspace(-2,2,8), d = 1/(delta+1e-6)
  spline = einsum('big,oig->bo', basis, spline_w)
  out    = base + spline        [B, OUT=1024]

Implementation:
  - Data parallel over tokens: 8 cores x 1024 tokens each; weights replicated.
  - The spline contraction is a [tok, IN*G=8192] @ [8192, OUT] matmul with K
    accumulated in PSUM (fp32). Mixed precision over the grid dimension:
      * inner grids g in {2,3,4,5} (|grid| <= 0.86, ~88% of the spline energy
        under x~N(0,1)): bf16 operands, 32 k-tiles per group.
      * outer grids g in {0,1,6,7}: fp8 e4m3 with DoubleRow perf mode (2
        k-tiles contracted per matmul), 16 pair-steps per group. Their small
        basis mass keeps the fp8 quantization error ~1.5e-2 total. Weights are
        scaled x4 host-side (out of the e4m3 denormal range); the matching
        1/4 on the basis is folded into the Exp activation bias.
  - Basis tiles are produced on the fly:
      v = (x - 2g)*x          (one scalar_tensor_tensor, fp32; VectorE, with
                               the fp8-pair second tile on GpSimd/Pool)
      basis = Exp(-d^2*v - d^2*g^2 [- ln 4])   (ScalarE, bf16/fp8 out)
    which equals exp(-d^2 (x-g)^2) [/4] exactly.
  - silu(x) is computed as x*(1+tanh(x/2)): tanh on ScalarE (same ACT table
    set as exp), the multiply-add on GpSimd/Pool; 0.5 folded into base_w.
  - base_b is added via a K=1 rank-1 matmul (ones row x bias row); the ones
    row is memset in the pre-tile preamble so HAM-warmup matmuls start as
    soon as the PE preamble finishes.
  - Per m-tile epilogue in EVERY group: base matmuls, bias, then immediate
    psum->sbuf eviction (DVE low half / ACT high half) + output DMA, keeping
    all psum banks free by the next group's first matmuls.
"""

import os
import sys

os.environ.setdefault("MYCRO_LOCAL_CACHE", "1")
for _p in ("/opt/trn_rl_repo", "/root/.axon_site/_ro/trn_rl_repo"):
    if os.path.isdir(_p) and _p not in sys.path:
        sys.path.insert(0, _p)

import numpy as np
import ml_dtypes

IN_F = 1024
OUT_F = 1024
G = 8
GRID_LO, GRID_HI = -2.0, 2.0
NCORES = 8
TOK = 8192
TCORE = TOK // NCORES   # 1024 tokens per core
NG = 2                  # token groups per core
GTOK = TCORE // NG      # 512 tokens per group
MT = GTOK // 128        # 4 psum m-tiles (128 tokens) per group
KB = IN_F // 128        # 8 k-tiles per grid / base k-tiles
WARMUP = 56             # HAM warmup matmuls

BF_G = (2, 3, 4, 5)     # bf16 grids (inner)
FP8_PAIRS = ((0, 1), (6, 7))  # fp8 DoubleRow grid pairs (outer)
K16 = len(BF_G) * KB    # 32 bf16 k-tiles
NQ8 = 2 * KB * 2        # 32 fp8 k-tiles (2 pairs x 8 i x 2 j)
FP8_SCALE = 4.0         # host: W*4; chip: basis/4 via exp bias

_DELTA = float((GRID_HI - GRID_LO) / (G - 1))
_D = 1.0 / (_DELTA + 1e-6)
# match jax's f32 linspace values
_GRID = np.linspace(GRID_LO, GRID_HI, G, dtype=np.float32).astype(np.float64)

TRACE = False
LAST_RESULT = None
_NC_CACHE = None


def build_nc(reps=1):
    from concourse import bacc
    import concourse.mybir as mybir
    import concourse.tile as tile

    F32 = mybir.dt.float32
    BF16 = mybir.dt.bfloat16
    F8 = mybir.dt.float8e4
    Alu = mybir.AluOpType
    Act = mybir.ActivationFunctionType
    DR = mybir.MatmulPerfMode.DoubleRow

    nc = bacc.Bacc("TRN2", target_bir_lowering=False)
    xg_d = nc.dram_tensor("xg", [NG, 128, KB, GTOK], F32, kind="ExternalInput")
    spl_d = nc.dram_tensor("spline", [K16 * 128, OUT_F], BF16, kind="ExternalInput")
    spl8_d = nc.dram_tensor("spline8", [NQ8 * 128, OUT_F], F8, kind="ExternalInput")
    bw_d = nc.dram_tensor("basew", [IN_F, OUT_F], BF16, kind="ExternalInput")
    bb_d = nc.dram_tensor("brow", [1, OUT_F], BF16, kind="ExternalInput")
    out_d = nc.dram_tensor("out", [TCORE, OUT_F], F32, kind="ExternalOutput")

    d2 = _D * _D

    # Register const APs for the per-grid Exp biases (activation() requires a
    # pre-registered [128,1] const tensor for non-trivial float biases).
    def register_const_ap(value):
        t = nc.alloc_sbuf_tensor(f"const-bias-{value}", [128, 1], F32)
        nc.gpsimd.memset(t.ap(), value)
        nc.const_aps.aps[(F32, value)] = t.ap()

    def exp_bias(g, fp8=False):
        gval = float(_GRID[g])
        b = -d2 * gval * gval
        if fp8:
            b -= float(np.log(FP8_SCALE))
        return float(b)

    bias_vals = {exp_bias(g) for g in BF_G}
    bias_vals |= {exp_bias(g, fp8=True) for p in FP8_PAIRS for g in p}
    for value in sorted(bias_vals):
        register_const_ap(value)
    # ones row for HAM warmup + rank-1 bias matmuls
    ones_t = nc.alloc_sbuf_tensor("ones-row", [1, 128], BF16)
    nc.gpsimd.memset(ones_t.ap(), 1.0)
    ones_ap = ones_t.ap()
    nc.all_engine_barrier()

    with tile.TileContext(nc) as tc:
        with (
            tc.tile_pool(name="const", bufs=1) as cpool,
            tc.tile_pool(name="xg", bufs=2) as xpool,
            tc.tile_pool(name="silu", bufs=1) as spool,
            tc.tile_pool(name="tanh", bufs=2) as tpool,
            tc.tile_pool(name="v", bufs=6) as vpool,
            tc.tile_pool(name="basis", bufs=6) as bpool,
            tc.tile_pool(name="b8", bufs=3) as b8pool,
            tc.tile_pool(name="osb", bufs=3) as opool,
            tc.tile_pool(name="psum", bufs=4, space="PSUM") as ppool,
        ):
            spl_sb = cpool.tile([128, K16, OUT_F], BF16)
            spl8_sb = cpool.tile([128, NQ8, OUT_F], F8)
            bw_sb = cpool.tile([128, KB, OUT_F], BF16)
            brow_sb = cpool.tile([1, OUT_F], BF16)
            spl_view = spl_d[:].rearrange("(k p) n -> p k n", p=128)
            spl8_view = spl8_d[:].rearrange("(k p) n -> p k n", p=128)
            bw_view = bw_d[:].rearrange("(k p) n -> p k n", p=128)

            if reps == 0:
                # minimal program used as a dispatch-overhead baseline
                z = cpool.tile([128, OUT_F], F32, name="zrow")
                nc.vector.memset(z[:], 0.0)
                nc.sync.dma_start(out_d[0:128, :], z[:])

            for rep in range(reps):
              for grp in range(NG):
                xg = xpool.tile([128, KB, GTOK], F32, tag="xg", name=f"xg_r{rep}g{grp}")
                ps = [
                    ppool.tile([128, OUT_F], F32, tag="ps", name=f"ps_g{grp}m{m}")
                    for m in range(MT)
                ]
                if grp == 0 and rep == 0:
                    # HAM warmup: keep the PE busy during the initial DMA wait
                    # so the first real matmuls run at 2.4GHz. Writes are
                    # discarded by the start=True of the first real matmul.
                    for w in range(WARMUP):
                        nc.tensor.matmul(
                            ps[w % MT][:, 0:128], ones_ap, ones_ap,
                            start=True, stop=True,
                        )
                if grp == 0:
                    # interleave the x blocks and the first spline k-tiles so
                    # the PE can start within a few us; then the bulk loads.
                    # The bf16 phase is i-major (4 k-tiles per x block), so
                    # each xg chunk unlocks the next 4 k-tiles.
                    nc.sync.dma_start(xg[:, 0:1, :], xg_d[grp, :, 0:1, :])
                    nc.sync.dma_start(spl_sb[:, 0:2, :], spl_view[:, 0:2, :])
                    nc.sync.dma_start(xg[:, 1:2, :], xg_d[grp, :, 1:2, :])
                    nc.sync.dma_start(spl_sb[:, 2:4, :], spl_view[:, 2:4, :])
                    nc.sync.dma_start(xg[:, 2:4, :], xg_d[grp, :, 2:4, :])
                    nc.sync.dma_start(spl_sb[:, 4:8, :], spl_view[:, 4:8, :])
                    nc.sync.dma_start(xg[:, 4:6, :], xg_d[grp, :, 4:6, :])
                    nc.sync.dma_start(spl_sb[:, 8:12, :], spl_view[:, 8:12, :])
                    nc.sync.dma_start(xg[:, 6:8, :], xg_d[grp, :, 6:8, :])
                    for c in range(3, 8):
                        nc.sync.dma_start(
                            spl_sb[:, c * 4:(c + 1) * 4, :],
                            spl_view[:, c * 4:(c + 1) * 4, :],
                        )
                    for c in range(4):
                        nc.sync.dma_start(
                            spl8_sb[:, c * 8:(c + 1) * 8, :],
                            spl8_view[:, c * 8:(c + 1) * 8, :],
                        )
                    nc.sync.dma_start(bw_sb[:], bw_view[:])
                    nc.sync.dma_start(brow_sb[:], bb_d[:])
                else:
                    nc.sync.dma_start(xg[:], xg_d[grp, :, :, :])
                silu = spool.tile([128, KB, GTOK], BF16)

                # ---- bf16 phase: inner grids, 32 k-tiles, i-major so each
                # xg chunk unlocks 4 consecutive k-tiles ----
                tanhs = [None] * KB
                for k16 in range(K16):
                    i = k16 // 4
                    g = BF_G[k16 % 4]
                    gval = float(_GRID[g])
                    v = vpool.tile([128, GTOK], F32)
                    nc.vector.scalar_tensor_tensor(
                        v[:], xg[:, i, :], -2.0 * gval, xg[:, i, :],
                        op0=Alu.add, op1=Alu.mult,
                    )
                    basis = bpool.tile([128, GTOK], BF16)
                    nc.scalar.activation(
                        basis[:], v[:], Act.Exp,
                        bias=exp_bias(g), scale=float(-d2),
                    )
                    for m in range(MT):
                        lhsT = basis[:, m * 128:(m + 1) * 128]
                        for n in range(2):
                            nc.tensor.matmul(
                                ps[m][:, n * 512:(n + 1) * 512],
                                lhsT,
                                spl_sb[:, k16, n * 512:(n + 1) * 512],
                                start=(k16 == 0), stop=False,
                            )
                    # silu2 = x*(1+tanh(x/2)) = 2*silu(x); 0.5 folded into
                    # basew. tanh right after block i's first exp (its xg is
                    # fresh); the multiply-add runs much later, in the second
                    # half of the phase, so the cross-engine tanh->stt->v
                    # latency chain never throttles basis production.
                    if k16 % 4 == 1:
                        j = k16 // 4
                        t = tpool.tile([128, GTOK], F32, tag="tanh")
                        nc.scalar.activation(t[:], xg[:, j, :], Act.Tanh, scale=0.5)
                        tanhs[j] = t
                        if j >= 1:
                            nc.vector.scalar_tensor_tensor(
                                silu[:, j - 1, :], tanhs[j - 1][:], 1.0,
                                xg[:, j - 1, :], op0=Alu.add, op1=Alu.mult,
                            )
                    if k16 == K16 - 1:
                        nc.vector.scalar_tensor_tensor(
                            silu[:, KB - 1, :], tanhs[KB - 1][:], 1.0,
                            xg[:, KB - 1, :], op0=Alu.add, op1=Alu.mult,
                        )

                # ---- fp8 phase: outer grid pairs, DoubleRow ----
                for step in range(2 * KB):
                    pi, i = divmod(step, KB)
                    b8 = b8pool.tile([128, 2, GTOK], F8)
                    for j in range(2):
                        g = FP8_PAIRS[pi][j]
                        gval = float(_GRID[g])
                        v = vpool.tile([128, GTOK], F32)
                        nc.vector.scalar_tensor_tensor(
                            v[:], xg[:, i, :], -2.0 * gval, xg[:, i, :],
                            op0=Alu.add, op1=Alu.mult,
                        )
                        nc.scalar.activation(
                            b8[:, j, :], v[:], Act.Exp,
                            bias=exp_bias(g, fp8=True), scale=float(-d2),
                        )
                    q = (pi * KB + i) * 2
                    for m in range(MT):
                        lhsT = b8[:, :, m * 128:(m + 1) * 128]
                        for n in range(2):
                            nc.tensor.matmul(
                                ps[m][:, n * 512:(n + 1) * 512],
                                lhsT,
                                spl8_sb[:, q:q + 2, n * 512:(n + 1) * 512],
                                start=False, stop=False,
                                perf_mode=DR,
                            )

                # ---- base phase: per m-tile base matmuls, bias, eviction ----
                for m in range(MT):
                    for kb in range(KB):
                        lhsT = silu[:, kb, m * 128:(m + 1) * 128]
                        for n in range(2):
                            nc.tensor.matmul(
                                ps[m][:, n * 512:(n + 1) * 512],
                                lhsT,
                                bw_sb[:, kb, n * 512:(n + 1) * 512],
                                start=False, stop=False,
                            )
                    for n in range(2):
                        nc.tensor.matmul(
                            ps[m][:, n * 512:(n + 1) * 512],
                            ones_ap,
                            brow_sb[0:1, n * 512:(n + 1) * 512],
                            start=False, stop=True,
                        )
                    mg = grp * MT + m
                    o = opool.tile([128, OUT_F], F32, tag="osb", name=f"o_{mg}")
                    nc.vector.tensor_copy(o[:, 0:512], ps[m][:, 0:512])
                    nc.scalar.copy(o[:, 512:1024], ps[m][:, 512:1024])
                    if grp == NG - 1 and m == MT - 1:
                        # shorten the tail: ship each half as soon as its copy
                        # is done
                        nc.sync.dma_start(
                            out_d[mg * 128:(mg + 1) * 128, 0:512], o[:, 0:512]
                        )
                        nc.sync.dma_start(
                            out_d[mg * 128:(mg + 1) * 128, 512:1024], o[:, 512:1024]
                        )
                    else:
                        nc.sync.dma_start(out_d[mg * 128:(mg + 1) * 128, :], o[:])

    nc.compile()
    return nc


def _host_prep(x, base_w, base_b, spline_w):
    x = np.asarray(x, dtype=np.float32)
    base_w = np.asarray(base_w, dtype=np.float32)
    base_b = np.asarray(base_b, dtype=np.float32)
    spline_w = np.asarray(spline_w, dtype=np.float32)

    x_flat = np.ascontiguousarray(x.reshape(TOK, IN_F))
    # [OUT, IN, G] -> [G, IN, OUT]; row of tile k is g*IN + i
    spl_gio = spline_w.transpose(2, 1, 0)  # [G, IN, OUT]
    # bf16 tiles are i-major: k16 = i*4 + g' with g' indexing BF_G
    spl16 = np.ascontiguousarray(
        spl_gio[list(BF_G)]
        .reshape(len(BF_G), KB, 128, OUT_F)
        .transpose(1, 0, 2, 3)
        .reshape(K16 * 128, OUT_F)
    ).astype(ml_dtypes.bfloat16)
    # fp8 part: [pair, i, j, 128, OUT] with j indexing the two grids of the
    # pair (DoubleRow contracts over the j dimension)
    blocks = []
    for (ga, gb) in FP8_PAIRS:
        a = spl_gio[ga].reshape(KB, 128, OUT_F)
        b = spl_gio[gb].reshape(KB, 128, OUT_F)
        blocks.append(np.stack([a, b], axis=1))  # [KB, 2, 128, OUT]
    spl8 = np.concatenate(blocks, axis=0).reshape(NQ8 * 128, OUT_F)
    spl8 = np.ascontiguousarray(spl8 * FP8_SCALE).astype(ml_dtypes.float8_e4m3)
    bw = np.ascontiguousarray(0.5 * base_w.T).astype(ml_dtypes.bfloat16)
    brow = np.ascontiguousarray(base_b.reshape(1, OUT_F)).astype(ml_dtypes.bfloat16)

    in_maps = []
    for c in range(NCORES):
        shard = x_flat[c * TCORE:(c + 1) * TCORE, :]   # [tok, in]
        xT = shard.T                                    # [in, tok]
        # [in, tok] -> [i, p, grp, t] -> [grp, p, i, t]
        xg = np.ascontiguousarray(
            xT.reshape(KB, 128, NG, GTOK).transpose(2, 1, 0, 3)
        )
        in_maps.append({
            "xg": xg, "spline": spl16, "spline8": spl8,
            "basew": bw, "brow": brow,
        })
    return in_maps


def kernel(x, base_w, base_b, spline_w):
    global _NC_CACHE, LAST_RESULT
    from concourse.bass_utils import run_bass_kernel_spmd

    in_maps = _host_prep(x, base_w, base_b, spline_w)
    if _NC_CACHE is None:
        _NC_CACHE = build_nc()
    res = run_bass_kernel_spmd(
        _NC_CACHE, in_maps, core_ids=list(range(NCORES)), trace=TRACE
    )
    LAST_RESULT = res
    outs = [np.asarray(r["out"]) for r in res.results]
    full = np.concatenate(outs, axis=0)  # [8192, 1024]
    return full.reshape(4, 2048, OUT_F)


# revision 13
# speedup vs baseline: 1.4945x; 1.0012x over previous
"""KANLinear (RBF-KAN) Trainium2 kernel.

Math (matches the reference):
  x_flat [B=8192, IN=1024]
  base   = silu(x) @ (base_w.T) + base_b
  basis[b,i,g] = exp(-(d*(x[b,i]-grid[g]))**2),  grid = linspace(-2,2,8), d = 1/(delta+1e-6)
  spline = einsum('big,oig->bo', basis, spline_w)
  out    = base + spline        [B, OUT=1024]

Implementation:
  - Data parallel over tokens: 8 cores x 1024 tokens each; weights replicated.
  - The spline contraction is a [tok, IN*G=8192] @ [8192, OUT] matmul with K
    accumulated in PSUM (fp32). Mixed precision over the grid dimension:
      * inner grids g in {2,3,4,5} (|grid| <= 0.86, ~88% of the spline energy
        under x~N(0,1)): bf16 operands, 32 k-tiles per group.
      * outer grids g in {0,1,6,7}: fp8 e4m3 with DoubleRow perf mode (2
        k-tiles contracted per matmul), 16 pair-steps per group. Their small
        basis mass keeps the fp8 quantization error ~1.5e-2 total. Weights are
        scaled x4 host-side (out of the e4m3 denormal range); the matching
        1/4 on the basis is folded into the Exp activation bias.
  - Basis tiles are produced on the fly:
      v = (x - 2g)*x          (one scalar_tensor_tensor, fp32; VectorE, with
                               the fp8-pair second tile on GpSimd/Pool)
      basis = Exp(-d^2*v - d^2*g^2 [- ln 4])   (ScalarE, bf16/fp8 out)
    which equals exp(-d^2 (x-g)^2) [/4] exactly.
  - silu(x) is computed as x*(1+tanh(x/2)): tanh on ScalarE (same ACT table
    set as exp), the multiply-add on GpSimd/Pool; 0.5 folded into base_w.
  - base_b is added via a K=1 rank-1 matmul (ones row x bias row); the ones
    row is memset in the pre-tile preamble so HAM-warmup matmuls start as
    soon as the PE preamble finishes.
  - Per m-tile epilogue in EVERY group: base matmuls, bias, then immediate
    psum->sbuf eviction (DVE low half / ACT high half) + output DMA, keeping
    all psum banks free by the next group's first matmuls.
"""

import os
import sys

os.environ.setdefault("MYCRO_LOCAL_CACHE", "1")
for _p in ("/opt/trn_rl_repo", "/root/.axon_site/_ro/trn_rl_repo"):
    if os.path.isdir(_p) and _p not in sys.path:
        sys.path.insert(0, _p)

import numpy as np
import ml_dtypes

IN_F = 1024
OUT_F = 1024
G = 8
GRID_LO, GRID_HI = -2.0, 2.0
NCORES = 8
TOK = 8192
TCORE = TOK // NCORES   # 1024 tokens per core
NG = 2                  # token groups per core
GTOK = TCORE // NG      # 512 tokens per group
MT = GTOK // 128        # 4 psum m-tiles (128 tokens) per group
KB = IN_F // 128        # 8 k-tiles per grid / base k-tiles
WARMUP = 56             # HAM warmup matmuls

BF_G = (2, 3, 4, 5)     # bf16 grids (inner)
FP8_PAIRS = ((0, 1), (6, 7))  # fp8 DoubleRow grid pairs (outer)
K16 = len(BF_G) * KB    # 32 bf16 k-tiles
NQ8 = 2 * KB * 2        # 32 fp8 k-tiles (2 pairs x 8 i x 2 j)
FP8_SCALE = 4.0         # host: W*4; chip: basis/4 via exp bias

_DELTA = float((GRID_HI - GRID_LO) / (G - 1))
_D = 1.0 / (_DELTA + 1e-6)
# match jax's f32 linspace values
_GRID = np.linspace(GRID_LO, GRID_HI, G, dtype=np.float32).astype(np.float64)

TRACE = False
LAST_RESULT = None
_NC_CACHE = None


def build_nc(reps=1):
    from concourse import bacc
    import concourse.mybir as mybir
    import concourse.tile as tile

    F32 = mybir.dt.float32
    BF16 = mybir.dt.bfloat16
    F8 = mybir.dt.float8e4
    Alu = mybir.AluOpType
    Act = mybir.ActivationFunctionType
    DR = mybir.MatmulPerfMode.DoubleRow

    nc = bacc.Bacc("TRN2", target_bir_lowering=False)
    xg_d = nc.dram_tensor("xg", [NG, 128, KB, GTOK], F32, kind="ExternalInput")
    spl_d = nc.dram_tensor("spline", [K16 * 128, OUT_F], BF16, kind="ExternalInput")
    spl8_d = nc.dram_tensor("spline8", [NQ8 * 128, OUT_F], F8, kind="ExternalInput")
    bw_d = nc.dram_tensor("basew", [IN_F, OUT_F], BF16, kind="ExternalInput")
    bb_d = nc.dram_tensor("brow", [1, OUT_F], BF16, kind="ExternalInput")
    out_d = nc.dram_tensor("out", [TCORE, OUT_F], F32, kind="ExternalOutput")

    d2 = _D * _D

    # Register const APs for the per-grid Exp biases (activation() requires a
    # pre-registered [128,1] const tensor for non-trivial float biases).
    def register_const_ap(value):
        t = nc.alloc_sbuf_tensor(f"const-bias-{value}", [128, 1], F32)
        nc.gpsimd.memset(t.ap(), value)
        nc.const_aps.aps[(F32, value)] = t.ap()

    def exp_bias(g, fp8=False):
        gval = float(_GRID[g])
        b = -d2 * gval * gval
        if fp8:
            b -= float(np.log(FP8_SCALE))
        return float(b)

    bias_vals = {exp_bias(g) for g in BF_G}
    bias_vals |= {exp_bias(g, fp8=True) for p in FP8_PAIRS for g in p}
    for value in sorted(bias_vals):
        register_const_ap(value)
    # ones row for HAM warmup + rank-1 bias matmuls
    ones_t = nc.alloc_sbuf_tensor("ones-row", [1, 128], BF16)
    nc.gpsimd.memset(ones_t.ap(), 1.0)
    ones_ap = ones_t.ap()
    nc.all_engine_barrier()

    with tile.TileContext(nc) as tc:
        with (
            tc.tile_pool(name="const", bufs=1) as cpool,
            tc.tile_pool(name="xg", bufs=2) as xpool,
            tc.tile_pool(name="silu", bufs=1) as spool,
            tc.tile_pool(name="tanh", bufs=2) as tpool,
            tc.tile_pool(name="v", bufs=6) as vpool,
            tc.tile_pool(name="basis", bufs=6) as bpool,
            tc.tile_pool(name="b8", bufs=3) as b8pool,
            tc.tile_pool(name="osb", bufs=3) as opool,
            tc.tile_pool(name="psum", bufs=4, space="PSUM") as ppool,
        ):
            spl_sb = cpool.tile([128, K16, OUT_F], BF16)
            spl8_sb = cpool.tile([128, NQ8, OUT_F], F8)
            bw_sb = cpool.tile([128, KB, OUT_F], BF16)
            brow_sb = cpool.tile([1, OUT_F], BF16)
            spl_view = spl_d[:].rearrange("(k p) n -> p k n", p=128)
            spl8_view = spl8_d[:].rearrange("(k p) n -> p k n", p=128)
            bw_view = bw_d[:].rearrange("(k p) n -> p k n", p=128)

            if reps == 0:
                # minimal program used as a dispatch-overhead baseline
                z = cpool.tile([128, OUT_F], F32, name="zrow")
                nc.vector.memset(z[:], 0.0)
                nc.sync.dma_start(out_d[0:128, :], z[:])

            for rep in range(reps):
              for grp in range(NG):
                xg = xpool.tile([128, KB, GTOK], F32, tag="xg", name=f"xg_r{rep}g{grp}")
                ps = [
                    ppool.tile([128, OUT_F], F32, tag="ps", name=f"ps_g{grp}m{m}")
                    for m in range(MT)
                ]
                if grp == 0 and rep == 0:
                    # HAM warmup: keep the PE busy during the initial DMA wait
                    # so the first real matmuls run at 2.4GHz. Writes are
                    # discarded by the start=True of the first real matmul.
                    for w in range(WARMUP):
                        nc.tensor.matmul(
                            ps[w % MT][:, 0:128], ones_ap, ones_ap,
                            start=True, stop=True,
                        )
                if grp == 0:
                    # interleave the x blocks with the spline tiles they
                    # unlock (per i-block: 4 bf16 k-tiles + 2 fp8 pairs), so
                    # the PE can start within a few us and never outruns DMA.
                    nc.sync.dma_start(xg[:, 0:1, :], xg_d[grp, :, 0:1, :])
                    nc.sync.dma_start(spl_sb[:, 0:2, :], spl_view[:, 0:2, :])
                    nc.sync.dma_start(xg[:, 1:2, :], xg_d[grp, :, 1:2, :])
                    nc.sync.dma_start(spl_sb[:, 2:4, :], spl_view[:, 2:4, :])
                    nc.sync.dma_start(spl8_sb[:, 0:4, :], spl8_view[:, 0:4, :])
                    nc.sync.dma_start(xg[:, 2:4, :], xg_d[grp, :, 2:4, :])
                    for i in range(1, KB):
                        nc.sync.dma_start(
                            spl_sb[:, i * 4:(i + 1) * 4, :],
                            spl_view[:, i * 4:(i + 1) * 4, :],
                        )
                        nc.sync.dma_start(
                            spl8_sb[:, i * 4:(i + 1) * 4, :],
                            spl8_view[:, i * 4:(i + 1) * 4, :],
                        )
                        if 3 + i < KB:
                            nc.sync.dma_start(
                                xg[:, 3 + i:4 + i, :], xg_d[grp, :, 3 + i:4 + i, :]
                            )
                    nc.sync.dma_start(bw_sb[:], bw_view[:])
                    nc.sync.dma_start(brow_sb[:], bb_d[:])
                else:
                    nc.sync.dma_start(xg[:], xg_d[grp, :, :, :])
                silu = spool.tile([128, KB, GTOK], BF16)

                # ---- spline: per i-block, 4 bf16 k-tiles then 2 fp8
                # DoubleRow pair-steps, so DVE/ACT load stays smooth ----
                tanhs = [None] * KB
                for i in range(KB):
                    for gi in range(4):
                        k16 = i * 4 + gi
                        g = BF_G[gi]
                        gval = float(_GRID[g])
                        v = vpool.tile([128, GTOK], F32)
                        nc.vector.scalar_tensor_tensor(
                            v[:], xg[:, i, :], -2.0 * gval, xg[:, i, :],
                            op0=Alu.add, op1=Alu.mult,
                        )
                        basis = bpool.tile([128, GTOK], BF16)
                        nc.scalar.activation(
                            basis[:], v[:], Act.Exp,
                            bias=exp_bias(g), scale=float(-d2),
                        )
                        for m in range(MT):
                            lhsT = basis[:, m * 128:(m + 1) * 128]
                            for n in range(2):
                                nc.tensor.matmul(
                                    ps[m][:, n * 512:(n + 1) * 512],
                                    lhsT,
                                    spl_sb[:, k16, n * 512:(n + 1) * 512],
                                    start=(k16 == 0), stop=False,
                                )
                        # silu2 = x*(1+tanh(x/2)) = 2*silu(x); 0.5 folded into
                        # basew. tanh right after block i's first exp (its xg
                        # is fresh); the multiply-add runs 4+ tiles later so
                        # the cross-engine tanh->stt->v chain never throttles
                        # basis production.
                        if gi == 1:
                            t = tpool.tile([128, GTOK], F32, tag="tanh")
                            nc.scalar.activation(
                                t[:], xg[:, i, :], Act.Tanh, scale=0.5
                            )
                            tanhs[i] = t
                            if i >= 1:
                                nc.vector.scalar_tensor_tensor(
                                    silu[:, i - 1, :], tanhs[i - 1][:], 1.0,
                                    xg[:, i - 1, :], op0=Alu.add, op1=Alu.mult,
                                )
                        if i == KB - 1 and gi == 3:
                            nc.vector.scalar_tensor_tensor(
                                silu[:, KB - 1, :], tanhs[KB - 1][:], 1.0,
                                xg[:, KB - 1, :], op0=Alu.add, op1=Alu.mult,
                            )
                    for pi in range(2):
                        b8 = b8pool.tile([128, 2, GTOK], F8)
                        for j in range(2):
                            g = FP8_PAIRS[pi][j]
                            gval = float(_GRID[g])
                            v = vpool.tile([128, GTOK], F32)
                            nc.vector.scalar_tensor_tensor(
                                v[:], xg[:, i, :], -2.0 * gval, xg[:, i, :],
                                op0=Alu.add, op1=Alu.mult,
                            )
                            nc.scalar.activation(
                                b8[:, j, :], v[:], Act.Exp,
                                bias=exp_bias(g, fp8=True), scale=float(-d2),
                            )
                        q = (i * 2 + pi) * 2
                        for m in range(MT):
                            lhsT = b8[:, :, m * 128:(m + 1) * 128]
                            for n in range(2):
                                nc.tensor.matmul(
                                    ps[m][:, n * 512:(n + 1) * 512],
                                    lhsT,
                                    spl8_sb[:, q:q + 2, n * 512:(n + 1) * 512],
                                    start=False, stop=False,
                                    perf_mode=DR,
                                )

                # ---- base phase: per m-tile base matmuls, bias, eviction ----
                for m in range(MT):
                    for kb in range(KB):
                        lhsT = silu[:, kb, m * 128:(m + 1) * 128]
                        for n in range(2):
                            nc.tensor.matmul(
                                ps[m][:, n * 512:(n + 1) * 512],
                                lhsT,
                                bw_sb[:, kb, n * 512:(n + 1) * 512],
                                start=False, stop=False,
                            )
                    for n in range(2):
                        nc.tensor.matmul(
                            ps[m][:, n * 512:(n + 1) * 512],
                            ones_ap,
                            brow_sb[0:1, n * 512:(n + 1) * 512],
                            start=False, stop=True,
                        )
                    mg = grp * MT + m
                    o = opool.tile([128, OUT_F], F32, tag="osb", name=f"o_{mg}")
                    nc.vector.tensor_copy(o[:, 0:512], ps[m][:, 0:512])
                    nc.scalar.copy(o[:, 512:1024], ps[m][:, 512:1024])
                    if grp == NG - 1 and m == MT - 1:
                        # shorten the tail: ship each half as soon as its copy
                        # is done
                        nc.sync.dma_start(
                            out_d[mg * 128:(mg + 1) * 128, 0:512], o[:, 0:512]
                        )
                        nc.sync.dma_start(
                            out_d[mg * 128:(mg + 1) * 128, 512:1024], o[:, 512:1024]
                        )
                    else:
                        nc.sync.dma_start(out_d[mg * 128:(mg + 1) * 128, :], o[:])

    nc.compile()
    return nc


def _host_prep(x, base_w, base_b, spline_w):
    x = np.asarray(x, dtype=np.float32)
    base_w = np.asarray(base_w, dtype=np.float32)
    base_b = np.asarray(base_b, dtype=np.float32)
    spline_w = np.asarray(spline_w, dtype=np.float32)

    x_flat = np.ascontiguousarray(x.reshape(TOK, IN_F))
    # [OUT, IN, G] -> [G, IN, OUT]; row of tile k is g*IN + i
    spl_gio = spline_w.transpose(2, 1, 0)  # [G, IN, OUT]
    # bf16 tiles are i-major: k16 = i*4 + g' with g' indexing BF_G
    spl16 = np.ascontiguousarray(
        spl_gio[list(BF_G)]
        .reshape(len(BF_G), KB, 128, OUT_F)
        .transpose(1, 0, 2, 3)
        .reshape(K16 * 128, OUT_F)
    ).astype(ml_dtypes.bfloat16)
    # fp8 part: [i, pair, j, 128, OUT] with j indexing the two grids of the
    # pair (DoubleRow contracts over the j dimension)
    pair_blocks = [
        np.stack(
            [spl_gio[ga].reshape(KB, 128, OUT_F), spl_gio[gb].reshape(KB, 128, OUT_F)],
            axis=1,
        )  # [KB, 2j, 128, OUT]
        for (ga, gb) in FP8_PAIRS
    ]
    spl8 = np.stack(pair_blocks, axis=1).reshape(NQ8 * 128, OUT_F)
    spl8 = np.ascontiguousarray(spl8 * FP8_SCALE).astype(ml_dtypes.float8_e4m3)
    bw = np.ascontiguousarray(0.5 * base_w.T).astype(ml_dtypes.bfloat16)
    brow = np.ascontiguousarray(base_b.reshape(1, OUT_F)).astype(ml_dtypes.bfloat16)

    in_maps = []
    for c in range(NCORES):
        shard = x_flat[c * TCORE:(c + 1) * TCORE, :]   # [tok, in]
        xT = shard.T                                    # [in, tok]
        # [in, tok] -> [i, p, grp, t] -> [grp, p, i, t]
        xg = np.ascontiguousarray(
            xT.reshape(KB, 128, NG, GTOK).transpose(2, 1, 0, 3)
        )
        in_maps.append({
            "xg": xg, "spline": spl16, "spline8": spl8,
            "basew": bw, "brow": brow,
        })
    return in_maps


def kernel(x, base_w, base_b, spline_w):
    global _NC_CACHE, LAST_RESULT
    from concourse.bass_utils import run_bass_kernel_spmd

    in_maps = _host_prep(x, base_w, base_b, spline_w)
    if _NC_CACHE is None:
        _NC_CACHE = build_nc()
    res = run_bass_kernel_spmd(
        _NC_CACHE, in_maps, core_ids=list(range(NCORES)), trace=TRACE
    )
    LAST_RESULT = res
    outs = [np.asarray(r["out"]) for r in res.results]
    full = np.concatenate(outs, axis=0)  # [8192, 1024]
    return full.reshape(4, 2048, OUT_F)
